# revision 13
# baseline (speedup 1.0000x reference)
"""CapsuleLayer (dynamic routing) Trainium2 Bass kernel.

Math (per example b):
  u_hat[b,i,o,n] = sum_v x[b,i,v] * W[i,o,v,n]        I=1152, O=10, V=8, N=16
  b_logits = 0; repeat n_routing times:
    c = softmax_o(b_logits); s = sum_i c*u_hat; out = squash(s)
    if not last: b_logits += sum_n u_hat*out

Distribution: batch B=256 sharded over 8 cores (32 each). W replicated.

Per-core layout (chunk = 8 examples, 4 chunks), i = ib*16 + il:
  K partitions k = il*8+v   (contraction rows of the u_hat matmul)
  M partitions p = b*16+il  (rows of u_hat / routing state)
  U[c] [128, 72, 160] bf16  u_hat,  U[(b,il), ib, (o,n)]
  xbd  [128, 18, 128] bf16  block-diag x stationary quarters (2 rotating bufs)
  cbd[c] [128, 72, 80] bf16 block-diag c stationary: CBD[(b,il), ib, (o,b')]
  w2   [128, 72, 160] bf16  W2[(il,v), ib, (o,n)] = W[ib*16+il, o, v, n]
  u_hat matmul (per ib): psum[(b,il'),(o,n)] = XBD[:,ib,:].T @ w2[:,ib,:]
  s matmul (per iter): psum[(o,b'),(o',n)] += CBD[:,ib,:].T @ U[:,ib,:]
    -> diagonal o==o' holds s[b', o, n]  (extracted via small DMAs)

Schedule: phase 1 computes u_hat for ALL 4 chunks up front (PE stays warm,
HBM loads double-buffered at quarter-chunk granularity); routing runs with
iterations OUTER and 4 chunks in flight, so chunk c+1's s-matmul (PE) and
softmax (ACT/GPSIMD) overlap chunk c's agreement (DVE, the bottleneck).
Work distribution per routing iter-chunk:
  PE    s-matmul (72 accumulating MMs, contiguous ib-major CBD slices)
  ACT   PSUM->SBUF evacuation, exp, small casts
  DVE   agreement product + bf16 add-tree (the bottleneck, ~12us), vrep
        shuffles, softmax reciprocal+normalize
  GPSIMD squash chain (rsqrt via bit hack + Newton), softmax o-reduction
  DMA   diag extraction, cbd block-diag scatter, output stores - spread
        round-robin over the sync/scalar/gpsimd queues (vector/tensor kept
        clean for the bottleneck engines)

SBUF is within ~1KB of full, so scratch is carved aggressively: all squash
temps live in slices of one [8, 248] tile (GPSIMD program order makes the
reuse race-free), the softmax o-sum tree is carved into the c2n/rs tiles via
bitcast views, and the agreement t4/t2t/final levels are carved back into
ph's storage after it is consumed.
"""

import os
import sys

import numpy as np

_TRN_REPO = "/opt/trn_rl_repo"
if _TRN_REPO not in sys.path:
    sys.path.insert(0, _TRN_REPO)

EPS = 1e-10
B, I, V, O, N = 256, 1152, 8, 10, 16
NCORES = 8
BLOC = B // NCORES          # 32 examples per core
BC = 8                      # examples per chunk
NCHUNK = BLOC // BC         # 4
IB = I // 16                # 72 i-blocks
IBH = IB // 2               # 36 (agreement half granularity)
IBQ = IB // 4               # 18 (xbd staging granularity)
ON = O * N                  # 160
RSQRT_MAGIC = 0x5F3759DF


def _build(n_routing: int):
    import concourse.bacc as bacc
    import concourse.tile as tile
    from concourse import mybir

    nc = bacc.Bacc("TRN2", target_bir_lowering=False, debug=False)
    f32 = mybir.dt.float32
    bf16 = mybir.dt.bfloat16

    xbdh = nc.dram_tensor(
        "xbdh", [NCHUNK, 128, IB, 128], bf16, kind="ExternalInput"
    )
    w2 = nc.dram_tensor("w2", [128, IB, ON], bf16, kind="ExternalInput")
    e2 = nc.dram_tensor("e2", [128, 80], bf16, kind="ExternalInput")
    out_d = nc.dram_tensor("out", [BLOC, O, N], f32, kind="ExternalOutput")

    with tile.TileContext(nc) as tc:
        with (
            tc.tile_pool(name="state", bufs=1) as state,
            tc.tile_pool(name="small", bufs=1) as small,
            tc.tile_pool(name="tree", bufs=1) as tree,
            tc.tile_pool(name="psA", bufs=4, space="PSUM") as psA,
            tc.tile_pool(name="psS", bufs=4, space="PSUM") as psS,
        ):
            Us = [
                state.tile([128, IB, ON], bf16, tag=f"U{j}", name=f"U{j}")
                for j in range(NCHUNK)
            ]
            cbds = [
                state.tile([128, IB, 80], bf16, tag=f"cbd{j}", name=f"cbd{j}")
                for j in range(NCHUNK)
            ] if n_routing > 1 else []
            # zero the block-diag background once; scatters only touch the
            # diagonal slots.  Spread across engines so nothing serializes.
            for j, cb in enumerate(cbds):
                if j == 0:
                    nc.vector.memset(cb[:], 0.0)
                elif j == 1:
                    nc.scalar.memzero(cb[:])
                else:
                    nc.gpsimd.memset(cb[:], 0.0)
            bbs = [
                state.tile([128, IB, O], bf16, tag=f"bb{j}", name=f"bb{j}")
                for j in range(NCHUNK)
            ] if n_routing > 1 else []
            e2s = state.tile([128, 80], bf16)
            nc.sync.dma_start(out=e2s[:], in_=e2[:])
            w2s = state.tile([128, IB, ON], bf16)
            nc.sync.dma_start(out=w2s[:], in_=w2[:])
            # single-buffer v replication tiles (only DVE/ACT touch them)
            v3b = state.tile([32, ON], bf16, name="v3b")
            if n_routing > 1:
                nc.vector.memset(v3b[:], 0.0)
            vrep = state.tile([128, ON], bf16, name="vrep")
            # squash constants (GPSIMD only runs tensor_tensor-class ops,
            # so scalars are materialized as [8, O] const regions)
            csq = state.tile([BC, 4 * O], f32, name="csq")
            nc.gpsimd.memset(csq[:, 0 * O:1 * O], 1.0)
            nc.gpsimd.memset(csq[:, 1 * O:2 * O], EPS)
            nc.gpsimd.memset(csq[:, 2 * O:3 * O], -0.5)
            nc.gpsimd.memset(csq[:, 3 * O:4 * O], 1.5)

            # ---------------- phase 1: u_hat for all chunks ----------------
            for c in range(NCHUNK):
                for h in range(4):
                    xbd = small.tile(
                        [128, IBQ, 128], bf16, tag="xbd", bufs=2, name="xbd"
                    )
                    nc.sync.dma_start(
                        out=xbd[:], in_=xbdh[c, :, h * IBQ:(h + 1) * IBQ, :]
                    )
                    for g in range(IBQ // 3):
                        ps = psA.tile([128, 3, ON], f32, tag="psA")
                        for j in range(3):
                            ib = h * IBQ + g * 3 + j
                            nc.tensor.matmul(
                                ps[:, j, :],
                                xbd[:, g * 3 + j, :],
                                w2s[:, ib, :],
                                start=True,
                                stop=True,
                            )
                        dst = Us[c][:, h * IBQ + g * 3:h * IBQ + (g + 1) * 3, :]
                        if g % 2 == 0:
                            nc.vector.tensor_copy(dst, ps[:])
                        else:
                            nc.scalar.copy(dst, ps[:])

            # ---------------- routing: iterations outer ----------------
            for it in range(n_routing):
                for c in range(NCHUNK):
                    _routing_iter(
                        nc, tc, mybir, small, tree, psS,
                        Us[c],
                        cbds[c] if cbds else None,
                        bbs[c] if bbs else None,
                        e2s, v3b, vrep, csq, out_d, c, it, n_routing,
                    )

    nc.compile()
    return nc


def _routing_iter(nc, tc, mybir, small, tree, psS, U, cbd, bb, e2s,
                  v3b, vrep, csq, out_d, c, it, n_routing):
    f32 = mybir.dt.float32
    bf16 = mybir.dt.bfloat16
    i32 = mybir.dt.int32
    AX = mybir.AxisListType
    OP = mybir.AluOpType
    AF = mybir.ActivationFunctionType
    g = nc.gpsimd

    dmaqs = [nc.sync, nc.scalar, nc.gpsimd]

    # s accumulation over i-blocks
    pss = psS.tile([80, ON], f32, tag="psS")
    for ib in range(IB):
        lhsT = e2s[:] if it == 0 else cbd[:, ib, :]
        nc.tensor.matmul(
            pss[:], lhsT, U[:, ib, :], start=(ib == 0), stop=(ib == IB - 1)
        )
    # PSUM -> SBUF, extract diag s[b, (o,n)] via DMAs (engine APs must start
    # at partition 0/32/64/96; DMAs are exempt from the base rule)
    sY = small.tile([80, ON], f32, tag="sY", bufs=1)
    nc.scalar.copy(sY[:], pss[:])
    s3 = small.tile([BC, ON], f32, tag="s3", bufs=2)
    for o in range(O):
        dmaqs[o % 3].dma_start(
            out=s3[:, o * N:(o + 1) * N],
            in_=sY[o * 8:(o + 1) * 8, o * N:(o + 1) * N],
        )
    # ---- squash, entirely on GPSIMD (frees DVE for the agreement) ----
    #   v3 = s3 * nsq * rsqrt(nse*(1+nsq)^2), fp32
    # All temps are slices of one scratch tile; GPSIMD's strict program
    # order makes the storage reuse race-free.
    sw = small.tile([BC, 248], f32, tag="sw", bufs=1, name="sw")
    sq = sw[:, 0:160].rearrange("b (o n) -> b o n", n=N)
    q8 = sw[:, 160:240].rearrange("b (o n) -> b o n", n=8)
    q4 = sw[:, 0:40].rearrange("b (o n) -> b o n", n=4)
    q2 = sw[:, 40:60].rearrange("b (o n) -> b o n", n=2)
    nsq = sw[:, 60:70]
    np1 = sw[:, 70:80]
    d1 = sw[:, 80:90]
    dd = sw[:, 90:100]
    ya = sw[:, 100:110]
    y2 = sw[:, 110:120]
    t2 = sw[:, 120:130]
    u2 = sw[:, 130:140]
    yb = sw[:, 140:150]
    cone = csq[:, 0 * O:1 * O]
    ceps = csq[:, 1 * O:2 * O]
    cnh = csq[:, 2 * O:3 * O]
    c15 = csq[:, 3 * O:4 * O]
    g.tensor_mul(sq, s3[:].rearrange("b (o n) -> b o n", n=N),
                 s3[:].rearrange("b (o n) -> b o n", n=N))
    g.tensor_add(q8, sq[:, :, 0:8], sq[:, :, 8:16])
    g.tensor_add(q4, q8[:, :, 0:4], q8[:, :, 4:8])
    g.tensor_add(q2, q4[:, :, 0:2], q4[:, :, 2:4])
    g.tensor_add(nsq, q2[:, :, 0], q2[:, :, 1])
    g.tensor_add(np1, nsq, cone)
    g.tensor_mul(d1, np1, np1)
    g.tensor_add(u2, nsq, ceps)     # u2 scratch: nse
    g.tensor_mul(dd, d1, u2)
    # rsqrt(dd): bit hack (int ops only run on DVE) + 2 Newton steps (GPSIMD)
    nc.vector.tensor_scalar(
        ya.bitcast(i32), dd.bitcast(i32), 1, None,
        op0=OP.logical_shift_right,
    )
    nc.vector.tensor_scalar(
        ya.bitcast(i32), ya.bitcast(i32), -1, RSQRT_MAGIC,
        op0=OP.mult, op1=OP.add,
    )
    yy, yn = ya, yb
    for _ in range(2):
        g.tensor_mul(y2, yy, yy)
        g.tensor_mul(t2, y2, dd)
        g.tensor_mul(y2, t2, cnh)   # y2 scratch: -0.5*y^2*dd
        g.tensor_add(u2, y2, c15)
        g.tensor_mul(yn, yy, u2)
        yy, yn = yn, yy
    sc = y2  # consumed; reuse for the squash scale
    g.tensor_mul(sc, nsq, yy)
    v3 = small.tile([BC, ON], f32, tag="v3", bufs=2)
    g.tensor_mul(
        v3[:].rearrange("b (o n) -> b o n", n=N),
        s3[:].rearrange("b (o n) -> b o n", n=N),
        sc.unsqueeze(2).broadcast_to([BC, O, N]),
    )

    if it == n_routing - 1:
        nc.scalar.dma_start(
            out=out_d[c * BC:(c + 1) * BC, :, :],
            in_=v3[:].rearrange("b (o n) -> b o n", n=N),
        )
        return

    # replicate v across il: vrep[(b,il), (o,n)] = v[b,o,n]
    nc.scalar.copy(v3b[0:BC, :], v3[:])
    for q in range(4):
        nc.vector.stream_shuffle(
            vrep[q * 32:(q + 1) * 32, :],
            v3b[:],
            [2 * q + (j // 16) for j in range(32)],
        )
    # agreement a[(b,il), ib, o] = sum_n U*vrep, 2 halves, n-reduce as a
    # bf16 add-tree on DVE (tensor_reduce runs 1x-only, the tree gets 2x).
    # t4/t2t/final levels are carved back into ph's storage (consumed).
    bcur = bb if it == 0 else small.tile(
        [128, IB, O], bf16, tag="bsum", bufs=1, name="bsum"
    )
    for h in range(2):
        ph = tree.tile([128, IBH, O, N], bf16, tag="ph", name="ph")
        nc.vector.tensor_mul(
            ph[:],
            U[:, h * IBH:(h + 1) * IBH, :].rearrange(
                "p i (o n) -> p i o n", n=N
            ),
            vrep[:]
            .rearrange("p (o n) -> p o n", n=N)
            .unsqueeze(1)
            .broadcast_to([128, IBH, O, N]),
        )
        t8 = tree.tile([128, IBH, O, 8], bf16, tag="t8", name="t8")
        nc.vector.tensor_add(t8[:], ph[:, :, :, 0:8], ph[:, :, :, 8:16])
        nc.vector.tensor_add(
            ph[:, :, :, 0:4], t8[:, :, :, 0:4], t8[:, :, :, 4:8]
        )
        nc.vector.tensor_add(
            ph[:, :, :, 4:6], ph[:, :, :, 0:2], ph[:, :, :, 2:4]
        )
        bslice = bcur[:, h * IBH:(h + 1) * IBH, :]
        if it == 0:
            nc.vector.tensor_add(bslice, ph[:, :, :, 4], ph[:, :, :, 5])
        else:
            nc.vector.tensor_add(
                ph[:, :, :, 6], ph[:, :, :, 4], ph[:, :, :, 5]
            )
            nc.vector.tensor_add(
                bslice, ph[:, :, :, 6], bb[:, h * IBH:(h + 1) * IBH, :]
            )
    if it != 0 and it < n_routing - 2:
        nc.vector.tensor_copy(bb[:], bcur[:])

    # softmax over o (contiguous innermost-o layout everywhere).  The o-sum
    # tree runs on GPSIMD; its levels are carved into the c2n / rs tiles.
    c2 = small.tile([128, IB, O], f32, tag="c2", bufs=1, name="c2")
    nc.scalar.activation(c2[:], bcur[:], AF.Exp)
    c2n = small.tile([128, IB, O], bf16, tag="c2n", bufs=1, name="c2n")
    e5 = c2n[:].bitcast(f32)  # [128, 72, 5] carved over c2n's bytes
    g.tensor_add(e5, c2[:, :, 0:5], c2[:, :, 5:10])
    e2t = small.tile([128, IB, 2], f32, tag="e2t", bufs=1, name="e2t")
    g.tensor_add(e2t[:], e5[:, :, 0:2], e5[:, :, 2:4])
    rs = small.tile([128, IB], f32, tag="rs", bufs=1, name="rs")
    e1 = rs[:]  # carved: e1 is consumed before rs is written
    g.tensor_add(e1, e2t[:, :, 0], e2t[:, :, 1])
    ssum = small.tile([128, IB], f32, tag="ssum", bufs=1, name="ssum")
    g.tensor_add(ssum[:], e1, e5[:, :, 4])
    nc.vector.reciprocal(rs[:], ssum[:])
    nc.vector.tensor_mul(
        c2n[:], c2[:], rs[:].unsqueeze(2).broadcast_to([128, IB, O])
    )
    # scatter diag to CBD[(b,il), ib, (o, b'=b)]
    for b in range(BC):
        dmaqs[b % 3].dma_start(
            out=cbd[b * 16:(b + 1) * 16, :, b:80:8],
            in_=c2n[b * 16:(b + 1) * 16, :, :],
        )


_CACHE = {}


def _get(n_routing: int):
    if n_routing not in _CACHE:
        _CACHE[n_routing] = _build(n_routing)
    return _CACHE[n_routing]


def _bf16(a):
    import ml_dtypes

    return np.asarray(a, dtype=ml_dtypes.bfloat16)


def _prep_host(inputs: np.ndarray, W: np.ndarray):
    x = np.ascontiguousarray(np.asarray(inputs, dtype=np.float32))
    W = np.asarray(W, dtype=np.float32)
    # w2[(il,v), ib, (o,n)] = W[ib*16+il, o, v, n]
    w2 = np.ascontiguousarray(
        W.reshape(IB, 16, O, V, N).transpose(1, 3, 0, 2, 4).reshape(128, IB, ON)
    )
    # e2[(b,il), (o,b')] = 0.1 * (b == b')   (uniform softmax weights)
    e2 = np.zeros((128, 80), dtype=np.float32)
    for b in range(8):
        e2[b * 16:(b + 1) * 16, np.arange(O) * 8 + b] = 0.1
    return x, _bf16(w2), _bf16(e2)


def _make_in_maps(inputs, W):
    x, w2, e2 = _prep_host(inputs, W)
    in_maps = []
    for core in range(NCORES):
        xc = x[core * BLOC:(core + 1) * BLOC]              # [32, 1152, 8]
        # xbdh[c, il*8+v, ib, b*16+il] = xc[c*BC+b, ib*16+il, v]
        xr = xc.reshape(NCHUNK, BC, IB, 16, V)
        xbdh = np.zeros((NCHUNK, 128, IB, 128), dtype=np.float32)
        for il in range(16):
            xbdh[:, il * 8:(il + 1) * 8, :, il::16] = xr[:, :, :, il, :].transpose(
                0, 3, 2, 1
            )
        in_maps.append({"xbdh": _bf16(xbdh), "w2": w2, "e2": e2})
    return in_maps


def kernel(inputs, W, n_routing):
    from concourse.bass_utils import run_bass_kernel_spmd

    n_routing = int(n_routing)
    nc = _get(n_routing)
    in_maps = _make_in_maps(inputs, W)
    res = run_bass_kernel_spmd(nc, in_maps, core_ids=list(range(NCORES)))
    outs = [res.results[i]["out"] for i in range(NCORES)]
    return np.concatenate(outs, axis=0).astype(np.float32)


# revision 15
# speedup vs baseline: 5.3937x; 5.3937x over previous
"""CapsuleLayer (dynamic routing) Trainium2 Bass kernel.

Math (per example b):
  u_hat[b,i,o,n] = sum_v x[b,i,v] * W[i,o,v,n]        I=1152, O=10, V=8, N=16
  b_logits = 0; repeat n_routing times:
    c = softmax_o(b_logits); s = sum_i c*u_hat; out = squash(s)
    if not last: b_logits += sum_n u_hat*out

Distribution: batch B=256 sharded over 8 cores (32 each). W replicated.

Per-core layout (chunk = 8 examples, 4 chunks), i = ib*16 + il:
  K partitions k = il*8+v   (contraction rows of the u_hat matmul)
  M partitions p = b*16+il  (rows of u_hat / routing state)
  U[c] [128, 72, 160] bf16  u_hat,  U[(b,il), ib, (o,n)]
  xbd  [128, 18, 128] bf16  block-diag x stationary quarters (2 rotating bufs)
  cbd[c] [128, 72, 80] bf16 block-diag c stationary: CBD[(b,il), ib, (o,b')]
  w2   [128, 72, 160] bf16  W2[(il,v), ib, (o,n)] = W[ib*16+il, o, v, n]
  u_hat matmul (per ib): psum[(b,il'),(o,n)] = XBD[:,ib,:].T @ w2[:,ib,:]
  s matmul (per iter): psum[(o,b'),(o',n)] += CBD[:,ib,:].T @ U[:,ib,:]
    -> diagonal o==o' holds s[b', o, n]  (extracted via small DMAs)

Schedule: phase 1 computes u_hat for ALL 4 chunks up front (PE stays warm,
HBM loads double-buffered at quarter-chunk granularity); routing runs with
iterations OUTER and 4 chunks in flight, so chunk c+1's s-matmul (PE) and
softmax (ACT/GPSIMD) overlap chunk c's agreement (DVE, the bottleneck).
Work distribution per routing iter-chunk:
  PE    s-matmul (72 accumulating MMs, contiguous ib-major CBD slices)
  ACT   PSUM->SBUF evacuation, exp, small casts
  DVE   agreement product + bf16 add-tree (the bottleneck, ~12us), vrep
        shuffles, softmax reciprocal+normalize
  GPSIMD squash chain (rsqrt via bit hack + Newton), softmax o-reduction
  DMA   diag extraction, cbd block-diag scatter, output stores - spread
        round-robin over the sync/scalar/gpsimd queues (vector/tensor kept
        clean for the bottleneck engines)

SBUF is within ~1KB of full, so scratch is carved aggressively: all squash
temps live in slices of one [8, 248] tile (GPSIMD program order makes the
reuse race-free), the softmax o-sum tree is carved into the c2n/rs tiles via
bitcast views, and the agreement t4/t2t/final levels are carved back into
ph's storage after it is consumed.
"""

import os
import sys

import numpy as np

_TRN_REPO = "/opt/trn_rl_repo"
if _TRN_REPO not in sys.path:
    sys.path.insert(0, _TRN_REPO)

EPS = 1e-10
B, I, V, O, N = 256, 1152, 8, 10, 16
NCORES = 8
BLOC = B // NCORES          # 32 examples per core
BC = 8                      # examples per chunk
NCHUNK = BLOC // BC         # 4
IB = I // 16                # 72 i-blocks
IBH = IB // 2               # 36 (agreement half granularity)
IBQ = IB // 4               # 18 (xbd staging granularity)
ON = O * N                  # 160
RSQRT_MAGIC = 0x5F3759DF


def _build(n_routing: int):
    import concourse.bacc as bacc
    import concourse.tile as tile
    from concourse import mybir

    nc = bacc.Bacc("TRN2", target_bir_lowering=False, debug=False)
    f32 = mybir.dt.float32
    bf16 = mybir.dt.bfloat16

    xbdh = nc.dram_tensor(
        "xbdh", [NCHUNK, 128, IB, 128], bf16, kind="ExternalInput"
    )
    w2 = nc.dram_tensor("w2", [128, IB, ON], bf16, kind="ExternalInput")
    e2 = nc.dram_tensor("e2", [128, 80], bf16, kind="ExternalInput")
    out_d = nc.dram_tensor("out", [BLOC, O, N], f32, kind="ExternalOutput")

    with tile.TileContext(nc) as tc:
        with (
            tc.tile_pool(name="state", bufs=1) as state,
            tc.tile_pool(name="small", bufs=1) as small,
            tc.tile_pool(name="tree", bufs=1) as tree,
            tc.tile_pool(name="psA", bufs=4, space="PSUM") as psA,
            tc.tile_pool(name="psS", bufs=4, space="PSUM") as psS,
        ):
            Us = [
                state.tile([128, IB, ON], bf16, tag=f"U{j}", name=f"U{j}")
                for j in range(NCHUNK)
            ]
            cbds = [
                state.tile([128, 80, IB], bf16, tag=f"cbd{j}", name=f"cbd{j}")
                for j in range(NCHUNK)
            ] if n_routing > 1 else []
            # zero the block-diag background once; scatters only touch the
            # diagonal slots.  Spread across engines so nothing serializes.
            for j, cb in enumerate(cbds):
                if j == 0:
                    nc.vector.memset(cb[:], 0.0)
                elif j == 1:
                    nc.scalar.memzero(cb[:])
                else:
                    nc.gpsimd.memset(cb[:], 0.0)
            bbs = [
                state.tile([128, IB, O], bf16, tag=f"bb{j}", name=f"bb{j}")
                for j in range(NCHUNK)
            ] if n_routing > 1 else []
            e2s = state.tile([128, 80], bf16)
            nc.sync.dma_start(out=e2s[:], in_=e2[:])
            w2s = state.tile([128, IB, ON], bf16)
            for q in range(4):
                nc.scalar.dma_start(
                    out=w2s[:, q * IBQ:(q + 1) * IBQ, :],
                    in_=w2[:, q * IBQ:(q + 1) * IBQ, :],
                )
            # single-buffer v replication tiles (only DVE/ACT touch them)
            v3b = state.tile([32, ON], bf16, name="v3b")
            if n_routing > 1:
                nc.vector.memset(v3b[:], 0.0)
            vrep = state.tile([128, ON], bf16, name="vrep")
            # squash constants (GPSIMD only runs tensor_tensor-class ops,
            # so scalars are materialized as [8, O] const regions)
            csq = state.tile([BC, 4 * O], f32, name="csq")
            nc.gpsimd.memset(csq[:, 0 * O:1 * O], 1.0)
            nc.gpsimd.memset(csq[:, 1 * O:2 * O], EPS)
            nc.gpsimd.memset(csq[:, 2 * O:3 * O], -0.5)
            nc.gpsimd.memset(csq[:, 3 * O:4 * O], 1.5)

            # ---------------- phase 1: u_hat for all chunks ----------------
            for c in range(NCHUNK):
                for h in range(4):
                    xbd = small.tile(
                        [128, IBQ, 128], bf16, tag="xbd", bufs=2, name="xbd"
                    )
                    nc.sync.dma_start(
                        out=xbd[:], in_=xbdh[c, :, h * IBQ:(h + 1) * IBQ, :]
                    )
                    for g in range(IBQ // 3):
                        ps = psA.tile([128, 3, ON], f32, tag="psA")
                        for j in range(3):
                            ib = h * IBQ + g * 3 + j
                            nc.tensor.matmul(
                                ps[:, j, :],
                                xbd[:, g * 3 + j, :],
                                w2s[:, ib, :],
                                start=True,
                                stop=True,
                            )
                        dst = Us[c][:, h * IBQ + g * 3:h * IBQ + (g + 1) * 3, :]
                        if g % 2 == 0:
                            nc.vector.tensor_copy(dst, ps[:])
                        else:
                            nc.scalar.copy(dst, ps[:])

            # ---------------- routing: iterations outer ----------------
            for it in range(n_routing):
                for c in range(NCHUNK):
                    _routing_iter(
                        nc, tc, mybir, small, tree, psS,
                        Us[c],
                        cbds[c] if cbds else None,
                        bbs[c] if bbs else None,
                        e2s, v3b, vrep, csq, out_d, c, it, n_routing,
                    )

    nc.compile()
    return nc


def _routing_iter(nc, tc, mybir, small, tree, psS, U, cbd, bb, e2s,
                  v3b, vrep, csq, out_d, c, it, n_routing):
    f32 = mybir.dt.float32
    bf16 = mybir.dt.bfloat16
    i32 = mybir.dt.int32
    AX = mybir.AxisListType
    OP = mybir.AluOpType
    AF = mybir.ActivationFunctionType
    g = nc.gpsimd

    dmaqs = [nc.sync, nc.scalar, nc.gpsimd]

    # s accumulation over i-blocks
    pss = psS.tile([80, ON], f32, tag="psS")
    for ib in range(IB):
        lhsT = e2s[:] if it == 0 else cbd[:, :, ib]
        nc.tensor.matmul(
            pss[:], lhsT, U[:, ib, :], start=(ib == 0), stop=(ib == IB - 1)
        )
    # PSUM -> SBUF, extract diag s[b, (o,n)] via DMAs (engine APs must start
    # at partition 0/32/64/96; DMAs are exempt from the base rule)
    sY = small.tile([80, ON], f32, tag="sY", bufs=1)
    nc.scalar.copy(sY[:], pss[:])
    s3 = small.tile([BC, ON], f32, tag="s3", bufs=2)
    for o in range(O):
        dmaqs[o % 3].dma_start(
            out=s3[:, o * N:(o + 1) * N],
            in_=sY[o * 8:(o + 1) * 8, o * N:(o + 1) * N],
        )
    # ---- squash, entirely on GPSIMD (frees DVE for the agreement) ----
    #   v3 = s3 * nsq * rsqrt(nse*(1+nsq)^2), fp32
    # All temps are slices of one scratch tile; GPSIMD's strict program
    # order makes the storage reuse race-free.
    sw = small.tile([BC, 248], f32, tag="sw", bufs=1, name="sw")
    sq = sw[:, 0:160].rearrange("b (o n) -> b o n", n=N)
    q8 = sw[:, 160:240].rearrange("b (o n) -> b o n", n=8)
    q4 = sw[:, 0:40].rearrange("b (o n) -> b o n", n=4)
    q2 = sw[:, 40:60].rearrange("b (o n) -> b o n", n=2)
    nsq = sw[:, 60:70]
    np1 = sw[:, 70:80]
    d1 = sw[:, 80:90]
    dd = sw[:, 90:100]
    ya = sw[:, 100:110]
    y2 = sw[:, 110:120]
    t2 = sw[:, 120:130]
    u2 = sw[:, 130:140]
    yb = sw[:, 140:150]
    cone = csq[:, 0 * O:1 * O]
    ceps = csq[:, 1 * O:2 * O]
    cnh = csq[:, 2 * O:3 * O]
    c15 = csq[:, 3 * O:4 * O]
    g.tensor_mul(sq, s3[:].rearrange("b (o n) -> b o n", n=N),
                 s3[:].rearrange("b (o n) -> b o n", n=N))
    g.tensor_add(q8, sq[:, :, 0:8], sq[:, :, 8:16])
    g.tensor_add(q4, q8[:, :, 0:4], q8[:, :, 4:8])
    g.tensor_add(q2, q4[:, :, 0:2], q4[:, :, 2:4])
    g.tensor_add(nsq, q2[:, :, 0], q2[:, :, 1])
    g.tensor_add(np1, nsq, cone)
    g.tensor_mul(d1, np1, np1)
    g.tensor_add(u2, nsq, ceps)     # u2 scratch: nse
    g.tensor_mul(dd, d1, u2)
    # rsqrt(dd): bit hack (int ops only run on DVE) + 2 Newton steps (GPSIMD)
    nc.vector.tensor_scalar(
        ya.bitcast(i32), dd.bitcast(i32), 1, None,
        op0=OP.logical_shift_right,
    )
    nc.vector.tensor_scalar(
        ya.bitcast(i32), ya.bitcast(i32), -1, RSQRT_MAGIC,
        op0=OP.mult, op1=OP.add,
    )
    yy, yn = ya, yb
    for _ in range(2):
        g.tensor_mul(y2, yy, yy)
        g.tensor_mul(t2, y2, dd)
        g.tensor_mul(y2, t2, cnh)   # y2 scratch: -0.5*y^2*dd
        g.tensor_add(u2, y2, c15)
        g.tensor_mul(yn, yy, u2)
        yy, yn = yn, yy
    sc = y2  # consumed; reuse for the squash scale
    g.tensor_mul(sc, nsq, yy)
    v3 = small.tile([BC, ON], f32, tag="v3", bufs=2)
    g.tensor_mul(
        v3[:].rearrange("b (o n) -> b o n", n=N),
        s3[:].rearrange("b (o n) -> b o n", n=N),
        sc.unsqueeze(2).broadcast_to([BC, O, N]),
    )

    if it == n_routing - 1:
        nc.scalar.dma_start(
            out=out_d[c * BC:(c + 1) * BC, :, :],
            in_=v3[:].rearrange("b (o n) -> b o n", n=N),
        )
        return

    # replicate v across il: vrep[(b,il), (o,n)] = v[b,o,n]
    nc.scalar.copy(v3b[0:BC, :], v3[:])
    for q in range(4):
        nc.vector.stream_shuffle(
            vrep[q * 32:(q + 1) * 32, :],
            v3b[:],
            [2 * q + (j // 16) for j in range(32)],
        )
    # agreement a[(b,il), ib, o] = sum_n U*vrep, 2 halves, n-reduce as a
    # bf16 add-tree on DVE (tensor_reduce runs 1x-only, the tree gets 2x).
    # t4/t2t/final levels are carved back into ph's storage (consumed).
    bcur = bb if it == 0 else small.tile(
        [128, IB, O], bf16, tag="bsum", bufs=1, name="bsum"
    )
    for h in range(2):
        ph = tree.tile([128, IBH * ON], bf16, tag="ph", name="ph")
        phv = ph[:].rearrange("p (i o n) -> p i o n", o=O, n=N)
        t4v = ph[:, 0:IBH * O * 4].rearrange("p (i o n) -> p i o n", o=O, n=4)
        t2v = ph[:, IBH * O * 4:IBH * O * 6].rearrange(
            "p (i o n) -> p i o n", o=O, n=2
        )
        afv = ph[:, IBH * O * 6:IBH * O * 7].rearrange(
            "p (i o) -> p i o", o=O
        )
        nc.vector.tensor_mul(
            phv,
            U[:, h * IBH:(h + 1) * IBH, :].rearrange(
                "p i (o n) -> p i o n", n=N
            ),
            vrep[:]
            .rearrange("p (o n) -> p o n", n=N)
            .unsqueeze(1)
            .broadcast_to([128, IBH, O, N]),
        )
        t8 = tree.tile([128, IBH, O, 8], bf16, tag="t8", name="t8")
        nc.vector.tensor_add(t8[:], phv[:, :, :, 0:8], phv[:, :, :, 8:16])
        nc.vector.tensor_add(t4v, t8[:, :, :, 0:4], t8[:, :, :, 4:8])
        nc.vector.tensor_add(t2v, t4v[:, :, :, 0:2], t4v[:, :, :, 2:4])
        bslice = bcur[:, h * IBH:(h + 1) * IBH, :]
        if it == 0:
            nc.vector.tensor_add(bslice, t2v[:, :, :, 0], t2v[:, :, :, 1])
        else:
            nc.vector.tensor_add(afv, t2v[:, :, :, 0], t2v[:, :, :, 1])
            nc.vector.tensor_add(
                bslice, afv, bb[:, h * IBH:(h + 1) * IBH, :]
            )
    if it != 0 and it < n_routing - 2:
        nc.vector.tensor_copy(bb[:], bcur[:])

    # softmax over o (contiguous innermost-o layout everywhere).  The o-sum
    # tree runs on GPSIMD; its levels are carved into the c2n / rs tiles.
    c2 = small.tile([128, O, IB], f32, tag="c2", bufs=1, name="c2")
    nc.scalar.activation(c2[:].transpose([0, 2, 1]), bcur[:], AF.Exp)
    c2n = small.tile([128, O, IB], bf16, tag="c2n", bufs=1, name="c2n")
    e5 = (
        c2n[:].bitcast(f32)
        .rearrange("p a b -> p (a b)")
        .rearrange("p (o i) -> p o i", o=5, i=IB)
    )  # [128, 5, 72] carved over c2n's bytes
    g.tensor_add(e5, c2[:, 0:5, :], c2[:, 5:10, :])
    e2t = small.tile([128, 2, IB], f32, tag="e2t", bufs=1, name="e2t")
    g.tensor_add(e2t[:], e5[:, 0:2, :], e5[:, 2:4, :])
    rs = small.tile([128, IB], f32, tag="rs", bufs=1, name="rs")
    e1 = rs[:]  # carved: e1 is consumed before rs is written
    g.tensor_add(e1, e2t[:, 0, :], e2t[:, 1, :])
    ssum = small.tile([128, IB], f32, tag="ssum", bufs=1, name="ssum")
    g.tensor_add(ssum[:], e1, e5[:, 4, :])
    nc.vector.reciprocal(rs[:], ssum[:])
    nc.vector.tensor_mul(
        c2n[:], c2[:], rs[:].unsqueeze(1).broadcast_to([128, O, IB])
    )
    # scatter diag to CBD[(b,il), (o, b'=b), ib] (ib-contiguous runs)
    for b in range(BC):
        dmaqs[b % 3].dma_start(
            out=cbd[b * 16:(b + 1) * 16, b:80:8, :],
            in_=c2n[b * 16:(b + 1) * 16, :, :],
        )


_CACHE = {}


def _get(n_routing: int):
    if n_routing not in _CACHE:
        _CACHE[n_routing] = _build(n_routing)
    return _CACHE[n_routing]


def _bf16(a):
    import ml_dtypes

    return np.asarray(a, dtype=ml_dtypes.bfloat16)


def _prep_host(inputs: np.ndarray, W: np.ndarray):
    x = np.ascontiguousarray(np.asarray(inputs, dtype=np.float32))
    W = np.asarray(W, dtype=np.float32)
    # w2[(il,v), ib, (o,n)] = W[ib*16+il, o, v, n]
    w2 = np.ascontiguousarray(
        W.reshape(IB, 16, O, V, N).transpose(1, 3, 0, 2, 4).reshape(128, IB, ON)
    )
    # e2[(b,il), (o,b')] = 0.1 * (b == b')   (uniform softmax weights)
    e2 = np.zeros((128, 80), dtype=np.float32)
    for b in range(8):
        e2[b * 16:(b + 1) * 16, np.arange(O) * 8 + b] = 0.1
    return x, _bf16(w2), _bf16(e2)


def _make_in_maps(inputs, W):
    x, w2, e2 = _prep_host(inputs, W)
    in_maps = []
    for core in range(NCORES):
        xc = x[core * BLOC:(core + 1) * BLOC]              # [32, 1152, 8]
        # xbdh[c, il*8+v, ib, b*16+il] = xc[c*BC+b, ib*16+il, v]
        xr = xc.reshape(NCHUNK, BC, IB, 16, V)
        xbdh = np.zeros((NCHUNK, 128, IB, 128), dtype=np.float32)
        for il in range(16):
            xbdh[:, il * 8:(il + 1) * 8, :, il::16] = xr[:, :, :, il, :].transpose(
                0, 3, 2, 1
            )
        in_maps.append({"xbdh": _bf16(xbdh), "w2": w2, "e2": e2})
    return in_maps


def kernel(inputs, W, n_routing):
    from concourse.bass_utils import run_bass_kernel_spmd

    n_routing = int(n_routing)
    nc = _get(n_routing)
    in_maps = _make_in_maps(inputs, W)
    res = run_bass_kernel_spmd(nc, in_maps, core_ids=list(range(NCORES)))
    outs = [res.results[i]["out"] for i in range(NCORES)]
    return np.concatenate(outs, axis=0).astype(np.float32)


# revision 16
# speedup vs baseline: 5.5611x; 1.0310x over previous
"""CapsuleLayer (dynamic routing) Trainium2 Bass kernel.

Math (per example b):
  u_hat[b,i,o,n] = sum_v x[b,i,v] * W[i,o,v,n]        I=1152, O=10, V=8, N=16
  b_logits = 0; repeat n_routing times:
    c = softmax_o(b_logits); s = sum_i c*u_hat; out = squash(s)
    if not last: b_logits += sum_n u_hat*out

Distribution: batch B=256 sharded over 8 cores (32 each). W replicated.

Per-core layout (chunk = 8 examples, 4 chunks), i = ib*16 + il:
  K partitions k = il*8+v   (contraction rows of the u_hat matmul)
  M partitions p = b*16+il  (rows of u_hat / routing state)
  U[c] [128, 72, 160] bf16  u_hat,  U[(b,il), ib, (o,n)]
  xbd  [128, 18, 128] bf16  block-diag x stationary quarters (2 rotating bufs)
  cbd[c] [128, 72, 80] bf16 block-diag c stationary: CBD[(b,il), ib, (o,b')]
  w2   [128, 72, 160] bf16  W2[(il,v), ib, (o,n)] = W[ib*16+il, o, v, n]
  u_hat matmul (per ib): psum[(b,il'),(o,n)] = XBD[:,ib,:].T @ w2[:,ib,:]
  s matmul (per iter): psum[(o,b'),(o',n)] += CBD[:,ib,:].T @ U[:,ib,:]
    -> diagonal o==o' holds s[b', o, n]  (extracted via small DMAs)

Schedule: phase 1 computes u_hat for ALL 4 chunks up front (PE stays warm,
HBM loads double-buffered at quarter-chunk granularity); routing runs with
iterations OUTER and 4 chunks in flight, so chunk c+1's s-matmul (PE) and
softmax (ACT/GPSIMD) overlap chunk c's agreement (DVE, the bottleneck).
Work distribution per routing iter-chunk:
  PE    s-matmul (72 accumulating MMs, contiguous ib-major CBD slices)
  ACT   PSUM->SBUF evacuation, exp, small casts
  DVE   agreement product + bf16 add-tree (the bottleneck, ~12us), vrep
        shuffles, softmax reciprocal+normalize
  GPSIMD squash chain (rsqrt via bit hack + Newton), softmax o-reduction
  DMA   diag extraction, cbd block-diag scatter, output stores - spread
        round-robin over the sync/scalar/gpsimd queues (vector/tensor kept
        clean for the bottleneck engines)

SBUF is within ~1KB of full, so scratch is carved aggressively: all squash
temps live in slices of one [8, 248] tile (GPSIMD program order makes the
reuse race-free), the softmax o-sum tree is carved into the c2n/rs tiles via
bitcast views, and the agreement t4/t2t/final levels are carved back into
ph's storage after it is consumed.
"""

import os
import sys

import numpy as np

_TRN_REPO = "/opt/trn_rl_repo"
if _TRN_REPO not in sys.path:
    sys.path.insert(0, _TRN_REPO)

EPS = 1e-10
B, I, V, O, N = 256, 1152, 8, 10, 16
NCORES = 8
BLOC = B // NCORES          # 32 examples per core
BC = 8                      # examples per chunk
NCHUNK = BLOC // BC         # 4
IB = I // 16                # 72 i-blocks
IBH = IB // 2               # 36 (agreement half granularity)
IBQ = IB // 4               # 18 (xbd staging granularity)
ON = O * N                  # 160
RSQRT_MAGIC = 0x5F3759DF


def _build(n_routing: int):
    import concourse.bacc as bacc
    import concourse.tile as tile
    from concourse import mybir

    nc = bacc.Bacc("TRN2", target_bir_lowering=False, debug=False)
    f32 = mybir.dt.float32
    bf16 = mybir.dt.bfloat16

    xbdh = nc.dram_tensor(
        "xbdh", [NCHUNK, 128, IB, 128], bf16, kind="ExternalInput"
    )
    w2 = nc.dram_tensor("w2", [128, IB, ON], bf16, kind="ExternalInput")
    e2 = nc.dram_tensor("e2", [128, 80], bf16, kind="ExternalInput")
    out_d = nc.dram_tensor("out", [BLOC, O, N], f32, kind="ExternalOutput")

    with tile.TileContext(nc) as tc:
        with (
            tc.tile_pool(name="state", bufs=1) as state,
            tc.tile_pool(name="small", bufs=1) as small,
            tc.tile_pool(name="tree", bufs=1) as tree,
            tc.tile_pool(name="psA", bufs=4, space="PSUM") as psA,
            tc.tile_pool(name="psS", bufs=4, space="PSUM") as psS,
        ):
            Us = [
                state.tile([128, IB, ON], bf16, tag=f"U{j}", name=f"U{j}")
                for j in range(NCHUNK)
            ]
            cbds = [
                state.tile([128, 80, IB], bf16, tag=f"cbd{j}", name=f"cbd{j}")
                for j in range(NCHUNK)
            ] if n_routing > 1 else []
            # zero the block-diag background once; scatters only touch the
            # diagonal slots.  Spread across engines so nothing serializes.
            for j, cb in enumerate(cbds):
                if j == 0:
                    nc.vector.memset(cb[:], 0.0)
                elif j == 1:
                    nc.scalar.memzero(cb[:])
                else:
                    nc.gpsimd.memset(cb[:], 0.0)
            bbs = [
                state.tile([128, IB, O], bf16, tag=f"bb{j}", name=f"bb{j}")
                for j in range(NCHUNK)
            ] if n_routing > 1 else []
            e2s = state.tile([128, 80], bf16)
            nc.sync.dma_start(out=e2s[:], in_=e2[:])
            w2s = state.tile([128, IB, ON], bf16)
            for q in range(4):
                nc.scalar.dma_start(
                    out=w2s[:, q * IBQ:(q + 1) * IBQ, :],
                    in_=w2[:, q * IBQ:(q + 1) * IBQ, :],
                )
            # single-buffer v replication tiles (only DVE/ACT touch them)
            v3b = state.tile([32, ON], bf16, name="v3b")
            if n_routing > 1:
                nc.vector.memset(v3b[:], 0.0)
            vrep = state.tile([128, ON], bf16, name="vrep")
            # squash constants (GPSIMD only runs tensor_tensor-class ops,
            # so scalars are materialized as [8, O] const regions)
            csq = state.tile([BC, 2 * O], f32, name="csq")
            nc.gpsimd.memset(csq[:, 0 * O:1 * O], 1.0)
            nc.gpsimd.memset(csq[:, 1 * O:2 * O], EPS)

            # ---------------- phase 1: u_hat for all chunks ----------------
            for c in range(NCHUNK):
                for h in range(4):
                    xbd = small.tile(
                        [128, IBQ, 128], bf16, tag="xbd", bufs=2, name="xbd"
                    )
                    nc.sync.dma_start(
                        out=xbd[:], in_=xbdh[c, :, h * IBQ:(h + 1) * IBQ, :]
                    )
                    for g in range(IBQ // 3):
                        ps = psA.tile([128, 3, ON], f32, tag="psA")
                        for j in range(3):
                            ib = h * IBQ + g * 3 + j
                            nc.tensor.matmul(
                                ps[:, j, :],
                                xbd[:, g * 3 + j, :],
                                w2s[:, ib, :],
                                start=True,
                                stop=True,
                            )
                        dst = Us[c][:, h * IBQ + g * 3:h * IBQ + (g + 1) * 3, :]
                        if g % 2 == 0:
                            nc.vector.tensor_copy(dst, ps[:])
                        else:
                            nc.scalar.copy(dst, ps[:])

            # ---------------- routing: iterations outer ----------------
            for it in range(n_routing):
                for c in range(NCHUNK):
                    _routing_iter(
                        nc, tc, mybir, small, tree, psS,
                        Us[c],
                        cbds[c] if cbds else None,
                        bbs[c] if bbs else None,
                        e2s, v3b, vrep, csq, out_d, c, it, n_routing,
                    )

    nc.compile()
    return nc


def _routing_iter(nc, tc, mybir, small, tree, psS, U, cbd, bb, e2s,
                  v3b, vrep, csq, out_d, c, it, n_routing):
    f32 = mybir.dt.float32
    bf16 = mybir.dt.bfloat16
    i32 = mybir.dt.int32
    AX = mybir.AxisListType
    OP = mybir.AluOpType
    AF = mybir.ActivationFunctionType
    g = nc.gpsimd

    dmaqs = [nc.sync, nc.scalar, nc.gpsimd]

    # s accumulation over i-blocks
    pss = psS.tile([80, ON], f32, tag="psS")
    for ib in range(IB):
        lhsT = e2s[:] if it == 0 else cbd[:, :, ib]
        nc.tensor.matmul(
            pss[:], lhsT, U[:, ib, :], start=(ib == 0), stop=(ib == IB - 1)
        )
    # PSUM -> SBUF, extract diag s[b, (o,n)] via DMAs (engine APs must start
    # at partition 0/32/64/96; DMAs are exempt from the base rule)
    sY = small.tile([80, ON], f32, tag="sY", bufs=1)
    nc.scalar.copy(sY[:], pss[:])
    s3 = small.tile([BC, ON], f32, tag="s3", bufs=2)
    for o in range(O):
        dmaqs[o % 3].dma_start(
            out=s3[:, o * N:(o + 1) * N],
            in_=sY[o * 8:(o + 1) * 8, o * N:(o + 1) * N],
        )
    # ---- squash, entirely on GPSIMD (frees DVE for the agreement) ----
    #   v3 = s3 * nsq * rsqrt(nse*(1+nsq)^2), fp32
    # All temps are slices of one scratch tile; GPSIMD's strict program
    # order makes the storage reuse race-free.
    sw = small.tile([BC, 248], f32, tag="sw", bufs=1, name="sw")
    sq = sw[:, 0:160].rearrange("b (o n) -> b o n", n=N)
    q8 = sw[:, 160:240].rearrange("b (o n) -> b o n", n=8)
    q4 = sw[:, 0:40].rearrange("b (o n) -> b o n", n=4)
    q2 = sw[:, 40:60].rearrange("b (o n) -> b o n", n=2)
    nsq = sw[:, 60:70]
    np1 = sw[:, 70:80]
    d1 = sw[:, 80:90]
    dd = sw[:, 90:100]
    ya = sw[:, 100:110]
    y2 = sw[:, 110:120]
    t2 = sw[:, 120:130]
    u2 = sw[:, 130:140]
    yb = sw[:, 140:150]
    cone = csq[:, 0 * O:1 * O]
    ceps = csq[:, 1 * O:2 * O]
    g.tensor_mul(sq, s3[:].rearrange("b (o n) -> b o n", n=N),
                 s3[:].rearrange("b (o n) -> b o n", n=N))
    g.tensor_add(q8, sq[:, :, 0:8], sq[:, :, 8:16])
    g.tensor_add(q4, q8[:, :, 0:4], q8[:, :, 4:8])
    g.tensor_add(q2, q4[:, :, 0:2], q4[:, :, 2:4])
    g.tensor_add(nsq, q2[:, :, 0], q2[:, :, 1])
    g.tensor_add(np1, nsq, cone)
    g.tensor_mul(d1, np1, np1)
    g.tensor_add(u2, nsq, ceps)     # u2 scratch: nse
    g.tensor_mul(dd, d1, u2)
    # rsqrt(dd) = exp(-0.5*ln(dd)) on ACT (Ln and Exp share a table set)
    nc.scalar.activation(ya, dd, AF.Ln)
    nc.scalar.activation(yb, ya, AF.Exp, scale=-0.5)
    sc = y2  # scratch slot for the squash scale
    g.tensor_mul(sc, nsq, yb)
    v3 = small.tile([BC, ON], f32, tag="v3", bufs=2)
    g.tensor_mul(
        v3[:].rearrange("b (o n) -> b o n", n=N),
        s3[:].rearrange("b (o n) -> b o n", n=N),
        sc.unsqueeze(2).broadcast_to([BC, O, N]),
    )

    if it == n_routing - 1:
        nc.scalar.dma_start(
            out=out_d[c * BC:(c + 1) * BC, :, :],
            in_=v3[:].rearrange("b (o n) -> b o n", n=N),
        )
        return

    # replicate v across il: vrep[(b,il), (o,n)] = v[b,o,n]
    nc.scalar.copy(v3b[0:BC, :], v3[:])
    for q in range(4):
        nc.vector.stream_shuffle(
            vrep[q * 32:(q + 1) * 32, :],
            v3b[:],
            [2 * q + (j // 16) for j in range(32)],
        )
    # agreement a[(b,il), ib, o] = sum_n U*vrep, 2 halves, n-reduce as a
    # bf16 add-tree on DVE (tensor_reduce runs 1x-only, the tree gets 2x).
    # t4/t2t/final levels are carved back into ph's storage (consumed).
    bcur = bb if it == 0 else small.tile(
        [128, IB, O], bf16, tag="bsum", bufs=1, name="bsum"
    )
    for h in range(2):
        ph = tree.tile([128, IBH * ON], bf16, tag="ph", name="ph")
        phv = ph[:].rearrange("p (i o n) -> p i o n", o=O, n=N)
        t4v = ph[:, 0:IBH * O * 4].rearrange("p (i o n) -> p i o n", o=O, n=4)
        t2v = ph[:, IBH * O * 4:IBH * O * 6].rearrange(
            "p (i o n) -> p i o n", o=O, n=2
        )
        afv = ph[:, IBH * O * 6:IBH * O * 7].rearrange(
            "p (i o) -> p i o", o=O
        )
        nc.vector.tensor_mul(
            phv,
            U[:, h * IBH:(h + 1) * IBH, :].rearrange(
                "p i (o n) -> p i o n", n=N
            ),
            vrep[:]
            .rearrange("p (o n) -> p o n", n=N)
            .unsqueeze(1)
            .broadcast_to([128, IBH, O, N]),
        )
        t8 = tree.tile([128, IBH, O, 8], bf16, tag="t8", name="t8")
        nc.vector.tensor_add(t8[:], phv[:, :, :, 0:8], phv[:, :, :, 8:16])
        nc.vector.tensor_add(t4v, t8[:, :, :, 0:4], t8[:, :, :, 4:8])
        nc.vector.tensor_add(t2v, t4v[:, :, :, 0:2], t4v[:, :, :, 2:4])
        bslice = bcur[:, h * IBH:(h + 1) * IBH, :]
        if it == 0:
            nc.vector.tensor_add(bslice, t2v[:, :, :, 0], t2v[:, :, :, 1])
        else:
            nc.vector.tensor_add(afv, t2v[:, :, :, 0], t2v[:, :, :, 1])
            nc.vector.tensor_add(
                bslice, afv, bb[:, h * IBH:(h + 1) * IBH, :]
            )
    if it != 0 and it < n_routing - 2:
        nc.vector.tensor_copy(bb[:], bcur[:])

    # softmax over o (contiguous innermost-o layout everywhere).  The o-sum
    # tree runs on GPSIMD; its levels are carved into the c2n / rs tiles.
    c2 = small.tile([128, O, IB], bf16, tag="c2", bufs=1, name="c2")
    nc.scalar.activation(c2[:].transpose([0, 2, 1]), bcur[:], AF.Exp)
    c2n = small.tile([128, O, IB], bf16, tag="c2n", bufs=1, name="c2n")
    e5 = (
        c2n[:].bitcast(f32)
        .rearrange("p a b -> p (a b)")
        .rearrange("p (o i) -> p o i", o=5, i=IB)
    )  # [128, 5, 72] carved over c2n's bytes
    g.tensor_add(e5, c2[:, 0:5, :], c2[:, 5:10, :])
    e2t = small.tile([128, 2, IB], f32, tag="e2t", bufs=1, name="e2t")
    g.tensor_add(e2t[:], e5[:, 0:2, :], e5[:, 2:4, :])
    rs = small.tile([128, IB], f32, tag="rs", bufs=1, name="rs")
    e1 = rs[:]  # carved: e1 is consumed before rs is written
    g.tensor_add(e1, e2t[:, 0, :], e2t[:, 1, :])
    ssum = small.tile([128, IB], f32, tag="ssum", bufs=1, name="ssum")
    g.tensor_add(ssum[:], e1, e5[:, 4, :])
    nc.vector.reciprocal(rs[:], ssum[:])
    nc.vector.tensor_mul(
        c2n[:], c2[:], rs[:].unsqueeze(1).broadcast_to([128, O, IB])
    )
    # scatter diag to CBD[(b,il), (o, b'=b), ib] (ib-contiguous runs)
    for b in range(BC):
        dmaqs[b % 3].dma_start(
            out=cbd[b * 16:(b + 1) * 16, b:80:8, :],
            in_=c2n[b * 16:(b + 1) * 16, :, :],
        )


_CACHE = {}


def _get(n_routing: int):
    if n_routing not in _CACHE:
        _CACHE[n_routing] = _build(n_routing)
    return _CACHE[n_routing]


def _bf16(a):
    import ml_dtypes

    return np.asarray(a, dtype=ml_dtypes.bfloat16)


def _prep_host(inputs: np.ndarray, W: np.ndarray):
    x = np.ascontiguousarray(np.asarray(inputs, dtype=np.float32))
    W = np.asarray(W, dtype=np.float32)
    # w2[(il,v), ib, (o,n)] = W[ib*16+il, o, v, n]
    w2 = np.ascontiguousarray(
        W.reshape(IB, 16, O, V, N).transpose(1, 3, 0, 2, 4).reshape(128, IB, ON)
    )
    # e2[(b,il), (o,b')] = 0.1 * (b == b')   (uniform softmax weights)
    e2 = np.zeros((128, 80), dtype=np.float32)
    for b in range(8):
        e2[b * 16:(b + 1) * 16, np.arange(O) * 8 + b] = 0.1
    return x, _bf16(w2), _bf16(e2)


def _make_in_maps(inputs, W):
    x, w2, e2 = _prep_host(inputs, W)
    in_maps = []
    for core in range(NCORES):
        xc = x[core * BLOC:(core + 1) * BLOC]              # [32, 1152, 8]
        # xbdh[c, il*8+v, ib, b*16+il] = xc[c*BC+b, ib*16+il, v]
        xr = xc.reshape(NCHUNK, BC, IB, 16, V)
        xbdh = np.zeros((NCHUNK, 128, IB, 128), dtype=np.float32)
        for il in range(16):
            xbdh[:, il * 8:(il + 1) * 8, :, il::16] = xr[:, :, :, il, :].transpose(
                0, 3, 2, 1
            )
        in_maps.append({"xbdh": _bf16(xbdh), "w2": w2, "e2": e2})
    return in_maps


def kernel(inputs, W, n_routing):
    from concourse.bass_utils import run_bass_kernel_spmd

    n_routing = int(n_routing)
    nc = _get(n_routing)
    in_maps = _make_in_maps(inputs, W)
    res = run_bass_kernel_spmd(nc, in_maps, core_ids=list(range(NCORES)))
    outs = [res.results[i]["out"] for i in range(NCORES)]
    return np.concatenate(outs, axis=0).astype(np.float32)


# revision 22
# speedup vs baseline: 6.0037x; 1.0796x over previous
"""CapsuleLayer (dynamic routing) Trainium2 Bass kernel.

Math (per example b):
  u_hat[b,i,o,n] = sum_v x[b,i,v] * W[i,o,v,n]        I=1152, O=10, V=8, N=16
  b_logits = 0; repeat n_routing times:
    c = softmax_o(b_logits); s = sum_i c*u_hat; out = squash(s)
    if not last: b_logits += sum_n u_hat*out

Distribution: batch B=256 sharded over 8 cores (32 each). W replicated.

Per-core layout (chunk = 8 examples, 4 chunks), i = ib*16 + il:
  K partitions k = il*8+v   (contraction rows of the u_hat matmul)
  M partitions p = b*16+il  (rows of u_hat / routing state)
  U[c] [128, 72, 160] bf16  u_hat,  U[(b,il), ib, (o,n)]
  xbd  [128, 18, 128] bf16  block-diag x stationary quarters (2 rotating bufs)
  cbd[c] [128, 72, 80] bf16 block-diag c stationary: CBD[(b,il), ib, (o,b')]
  w2   [128, 72, 160] bf16  W2[(il,v), ib, (o,n)] = W[ib*16+il, o, v, n]
  u_hat matmul (per ib): psum[(b,il'),(o,n)] = XBD[:,ib,:].T @ w2[:,ib,:]
  s matmul (per iter): psum[(o,b'),(o',n)] += CBD[:,ib,:].T @ U[:,ib,:]
    -> diagonal o==o' holds s[b', o, n]  (extracted via small DMAs)

Schedule: phase 1 computes u_hat for ALL 4 chunks up front (PE stays warm,
HBM loads double-buffered at quarter-chunk granularity); routing runs with
iterations OUTER and 4 chunks in flight, so chunk c+1's s-matmul (PE) and
softmax (ACT/GPSIMD) overlap chunk c's agreement (DVE, the bottleneck).
Work distribution per routing iter-chunk:
  PE    s-matmul (72 accumulating MMs, contiguous ib-major CBD slices)
  ACT   PSUM->SBUF evacuation, exp, small casts
  DVE   agreement product + bf16 add-tree (the bottleneck, ~12us), vrep
        shuffles, softmax reciprocal+normalize
  GPSIMD squash chain (rsqrt via bit hack + Newton), softmax o-reduction
  DMA   diag extraction, cbd block-diag scatter, output stores - spread
        round-robin over the sync/scalar/gpsimd queues (vector/tensor kept
        clean for the bottleneck engines)

SBUF is within ~1KB of full, so scratch is carved aggressively: all squash
temps live in slices of one [8, 248] tile (GPSIMD program order makes the
reuse race-free), the softmax o-sum tree is carved into the c2n/rs tiles via
bitcast views, and the agreement t4/t2t/final levels are carved back into
ph's storage after it is consumed.
"""

import os
import sys

import numpy as np

_TRN_REPO = "/opt/trn_rl_repo"
if _TRN_REPO not in sys.path:
    sys.path.insert(0, _TRN_REPO)

EPS = 1e-10
B, I, V, O, N = 256, 1152, 8, 10, 16
NCORES = 8
BLOC = B // NCORES          # 32 examples per core
BC = 8                      # examples per chunk
NCHUNK = BLOC // BC         # 4
IB = I // 16                # 72 i-blocks
IBH = IB // 2               # 36 (agreement half granularity)
IBQ = IB // 4               # 18 (xbd staging granularity)
ON = O * N                  # 160
RSQRT_MAGIC = 0x5F3759DF


def _build(n_routing: int):
    import concourse.bacc as bacc
    import concourse.tile as tile
    from concourse import mybir

    nc = bacc.Bacc("TRN2", target_bir_lowering=False, debug=False)
    f32 = mybir.dt.float32
    bf16 = mybir.dt.bfloat16

    xbdh = nc.dram_tensor(
        "xbdh", [NCHUNK, 128, IB, 128], bf16, kind="ExternalInput"
    )
    w2 = nc.dram_tensor("w2", [128, IB, ON], bf16, kind="ExternalInput")
    e2 = nc.dram_tensor("e2", [128, 80], bf16, kind="ExternalInput")
    out_d = nc.dram_tensor("out", [BLOC, O, N], f32, kind="ExternalOutput")

    with tile.TileContext(nc) as tc:
        with (
            tc.tile_pool(name="state", bufs=1) as state,
            tc.tile_pool(name="small", bufs=1) as small,
            tc.tile_pool(name="tree", bufs=1) as tree,
            tc.tile_pool(name="psA", bufs=4, space="PSUM") as psA,
            tc.tile_pool(name="psS", bufs=4, space="PSUM") as psS,
        ):
            Us = [
                state.tile([128, IB, ON], bf16, tag=f"U{j}", name=f"U{j}")
                for j in range(NCHUNK)
            ]
            cbds = [
                state.tile([128, 80, IB], bf16, tag=f"cbd{j}", name=f"cbd{j}")
                for j in range(NCHUNK)
            ] if n_routing > 1 else []
            # zero the block-diag background once; scatters only touch the
            # diagonal slots.  Spread across engines so nothing serializes.
            for j, cb in enumerate(cbds):
                if j % 2 == 0:
                    nc.scalar.memzero(cb[:])
                else:
                    nc.gpsimd.memset(cb[:], 0.0)
            bbs = [
                state.tile([128, IB, O], bf16, tag=f"bb{j}", name=f"bb{j}")
                for j in range(NCHUNK)
            ] if n_routing > 1 else []
            e2s = state.tile([128, 80], bf16)
            nc.sync.dma_start(out=e2s[:], in_=e2[:])
            w2s = state.tile([128, IB, ON], bf16)
            for q in range(4):
                nc.scalar.dma_start(
                    out=w2s[:, q * IBQ:(q + 1) * IBQ, :],
                    in_=w2[:, q * IBQ:(q + 1) * IBQ, :],
                )
            # single-buffer v replication tiles (only DVE/ACT touch them)
            v3b = state.tile([32, ON], bf16, name="v3b")
            if n_routing > 1:
                nc.vector.memset(v3b[:], 0.0)
            vrep = state.tile([128, ON], bf16, name="vrep")
            # squash constants (GPSIMD only runs tensor_tensor-class ops,
            # so scalars are materialized as [8, O] const regions)
            csq = state.tile([BC, 2 * O], f32, name="csq")
            nc.gpsimd.memset(csq[:, 0 * O:1 * O], 1.0)
            nc.gpsimd.memset(csq[:, 1 * O:2 * O], EPS)

            # ---------------- phase 1: u_hat for all chunks ----------------
            for c in range(NCHUNK):
                for h in range(4):
                    xbd = small.tile(
                        [128, IBQ, 128], bf16, tag="xbd", bufs=2, name="xbd"
                    )
                    nc.sync.dma_start(
                        out=xbd[:], in_=xbdh[c, :, h * IBQ:(h + 1) * IBQ, :]
                    )
                    for g in range(IBQ // 3):
                        ps = psA.tile([128, 3, ON], f32, tag="psA")
                        for j in range(3):
                            ib = h * IBQ + g * 3 + j
                            nc.tensor.matmul(
                                ps[:, j, :],
                                xbd[:, g * 3 + j, :],
                                w2s[:, ib, :],
                                start=True,
                                stop=True,
                            )
                        dst = Us[c][:, h * IBQ + g * 3:h * IBQ + (g + 1) * 3, :]
                        if g % 3 == 0:
                            nc.vector.tensor_copy(dst, ps[:])
                        else:
                            nc.scalar.copy(dst, ps[:])

            # ---------------- routing: iterations outer ----------------
            for it in range(n_routing):
                for c in range(NCHUNK):
                    _routing_iter(
                        nc, tc, mybir, small, tree, psS,
                        Us[c],
                        cbds[c] if cbds else None,
                        bbs[c] if bbs else None,
                        e2s, v3b, vrep, csq, out_d, c, it, n_routing,
                    )

    nc.compile()
    return nc


def _routing_iter(nc, tc, mybir, small, tree, psS, U, cbd, bb, e2s,
                  v3b, vrep, csq, out_d, c, it, n_routing):
    f32 = mybir.dt.float32
    bf16 = mybir.dt.bfloat16
    i32 = mybir.dt.int32
    AX = mybir.AxisListType
    OP = mybir.AluOpType
    AF = mybir.ActivationFunctionType
    g = nc.gpsimd

    dmaqs = [nc.sync, nc.scalar, nc.gpsimd]

    # s accumulation over i-blocks
    pss = psS.tile([80, ON], f32, tag="psS")
    for ib in range(IB):
        lhsT = e2s[:] if it == 0 else cbd[:, :, ib]
        nc.tensor.matmul(
            pss[:], lhsT, U[:, ib, :], start=(ib == 0), stop=(ib == IB - 1)
        )
    # PSUM -> SBUF, extract diag s[b, (o,n)] via DMAs (engine APs must start
    # at partition 0/32/64/96; DMAs are exempt from the base rule)
    sY = small.tile([80, ON], f32, tag="sY", bufs=1)
    nc.scalar.copy(sY[:], pss[:])
    s3 = small.tile([BC, ON], f32, tag="s3", bufs=2)
    for o in range(O):
        dmaqs[o % 3].dma_start(
            out=s3[:, o * N:(o + 1) * N],
            in_=sY[o * 8:(o + 1) * 8, o * N:(o + 1) * N],
        )
    # ---- squash: v3 = s3 * nsq * rsqrt(nse*(1+nsq)^2), fp32 ----
    # GPSIMD computes the front (s^2 and its n-tree + dd) while DVE is busy
    # with the previous chunk's agreement; the short rsqrt+scale chain runs
    # on DVE itself (cheaper than DVE idling on a slow GPSIMD Newton tail).
    # Temps are slices of one scratch tile; cross-chunk reuse is ordered by
    # each engine's program order plus the single GPSIMD->DVE handoff at dd.
    sw = small.tile([BC, 248], f32, tag="sw", bufs=1, name="sw")
    sq = sw[:, 0:160].rearrange("b (o n) -> b o n", n=N)
    q8 = sw[:, 160:240].rearrange("b (o n) -> b o n", n=8)
    q4 = sw[:, 0:40].rearrange("b (o n) -> b o n", n=4)
    q2 = sw[:, 40:60].rearrange("b (o n) -> b o n", n=2)
    nsq = sw[:, 60:70]
    np1 = sw[:, 70:80]
    d1 = sw[:, 80:90]
    dd = sw[:, 90:100]
    ya = sw[:, 100:110]
    y2 = sw[:, 110:120]
    w1 = sw[:, 120:130]
    yb = sw[:, 140:150]
    cone = csq[:, 0 * O:1 * O]
    ceps = csq[:, 1 * O:2 * O]
    g.tensor_mul(sq, s3[:].rearrange("b (o n) -> b o n", n=N),
                 s3[:].rearrange("b (o n) -> b o n", n=N))
    g.tensor_add(q8, sq[:, :, 0:8], sq[:, :, 8:16])
    g.tensor_add(q4, q8[:, :, 0:4], q8[:, :, 4:8])
    g.tensor_add(q2, q4[:, :, 0:2], q4[:, :, 2:4])
    g.tensor_add(nsq, q2[:, :, 0], q2[:, :, 1])
    g.tensor_add(np1, nsq, cone)
    g.tensor_mul(d1, np1, np1)
    g.tensor_add(y2, nsq, ceps)     # y2 scratch: nse
    g.tensor_mul(dd, d1, y2)
    # rsqrt(dd) on DVE: bit-hack seed + 2 STT-fused Newton steps
    v = nc.vector
    v.tensor_scalar(
        ya.bitcast(i32), dd.bitcast(i32), 1, None,
        op0=OP.logical_shift_right,
    )
    v.tensor_scalar(
        ya.bitcast(i32), ya.bitcast(i32), -1, RSQRT_MAGIC,
        op0=OP.mult, op1=OP.add,
    )
    yy, yn = ya, yb
    for _ in range(2):
        v.tensor_mul(y2, yy, yy)
        # w1 = (y2 * -0.5) * dd;  yn = (w1 + 1.5) * yy
        v.scalar_tensor_tensor(w1, y2, -0.5, dd, op0=OP.mult, op1=OP.mult)
        v.scalar_tensor_tensor(yn, w1, 1.5, yy, op0=OP.add, op1=OP.mult)
        yy, yn = yn, yy
    sc = y2  # consumed; reuse for the squash scale
    v.tensor_mul(sc, nsq, yy)
    v3 = small.tile([BC, ON], f32, tag="v3", bufs=2)
    v.tensor_mul(
        v3[:].rearrange("b (o n) -> b o n", n=N),
        s3[:].rearrange("b (o n) -> b o n", n=N),
        sc.unsqueeze(2).broadcast_to([BC, O, N]),
    )

    if it == n_routing - 1:
        nc.scalar.dma_start(
            out=out_d[c * BC:(c + 1) * BC, :, :],
            in_=v3[:].rearrange("b (o n) -> b o n", n=N),
        )
        return

    # replicate v across il: vrep[(b,il), (o,n)] = v[b,o,n]
    nc.vector.tensor_copy(v3b[0:BC, :], v3[:])
    for q in range(4):
        nc.vector.stream_shuffle(
            vrep[q * 32:(q + 1) * 32, :],
            v3b[:],
            [2 * q + (j // 16) for j in range(32)],
        )
    # agreement a[(b,il), ib, o] = sum_n U*vrep, 2 halves, n-reduce as a
    # bf16 add-tree on DVE (tensor_reduce runs 1x-only, the tree gets 2x).
    # t4/t2t/final levels are carved back into ph's storage (consumed).
    bcur = bb if it == 0 else small.tile(
        [128, IB, O], bf16, tag="bsum", bufs=1, name="bsum"
    )
    c2 = small.tile([128, O, IB], bf16, tag="c2", bufs=1, name="c2")
    for h in range(2):
        ph = tree.tile([128, IBH * ON], bf16, tag="ph", name="ph")
        phv = ph[:].rearrange("p (i o n) -> p i o n", o=O, n=N)
        t4v = ph[:, 0:IBH * O * 4].rearrange("p (i o n) -> p i o n", o=O, n=4)
        t2v = ph[:, IBH * O * 4:IBH * O * 6].rearrange(
            "p (i o n) -> p i o n", o=O, n=2
        )
        afv = ph[:, IBH * O * 6:IBH * O * 7].rearrange(
            "p (i o) -> p i o", o=O
        )
        nc.vector.tensor_mul(
            phv,
            U[:, h * IBH:(h + 1) * IBH, :].rearrange(
                "p i (o n) -> p i o n", n=N
            ),
            vrep[:]
            .rearrange("p (o n) -> p o n", n=N)
            .unsqueeze(1)
            .broadcast_to([128, IBH, O, N]),
        )
        t8 = tree.tile([128, IBH, O, 8], bf16, tag="t8", name="t8")
        nc.vector.tensor_add(t8[:], phv[:, :, :, 0:8], phv[:, :, :, 8:16])
        nc.vector.tensor_add(t4v, t8[:, :, :, 0:4], t8[:, :, :, 4:8])
        nc.vector.tensor_add(t2v, t4v[:, :, :, 0:2], t4v[:, :, :, 2:4])
        bslice = bcur[:, h * IBH:(h + 1) * IBH, :]
        if it == 0:
            nc.vector.tensor_add(bslice, t2v[:, :, :, 0], t2v[:, :, :, 1])
        else:
            nc.vector.tensor_add(afv, t2v[:, :, :, 0], t2v[:, :, :, 1])
            nc.vector.tensor_add(
                bslice, afv, bb[:, h * IBH:(h + 1) * IBH, :]
            )
        # exp of this half immediately on ACT, so the softmax input is
        # ready by the time DVE finishes the other half's tree
        nc.scalar.activation(
            c2[:, :, h * IBH:(h + 1) * IBH].transpose([0, 2, 1]),
            bslice, AF.Exp,
        )
    if it != 0 and it < n_routing - 2:
        nc.vector.tensor_copy(bb[:], bcur[:])

    # softmax over o: the o-sum tree runs on DVE at 2x (on GPSIMD it
    # stalled the reciprocal); levels are carved into the c2n / rs tiles.
    c2n = small.tile([128, O, IB], bf16, tag="c2n", bufs=1, name="c2n")
    e5 = (
        c2n[:].bitcast(f32)
        .rearrange("p a b -> p (a b)")
        .rearrange("p (o i) -> p o i", o=5, i=IB)
    )  # [128, 5, 72] carved over c2n's bytes
    nc.vector.tensor_add(e5, c2[:, 0:5, :], c2[:, 5:10, :])
    e2t = small.tile([128, 2, IB], f32, tag="e2t", bufs=1, name="e2t")
    nc.vector.tensor_add(e2t[:], e5[:, 0:2, :], e5[:, 2:4, :])
    rs = small.tile([128, IB], f32, tag="rs", bufs=1, name="rs")
    e1 = rs[:]  # carved: e1 is consumed before rs is written
    nc.vector.tensor_add(e1, e2t[:, 0, :], e2t[:, 1, :])
    ssum = small.tile([128, IB], f32, tag="ssum", bufs=1, name="ssum")
    nc.vector.tensor_add(ssum[:], e1, e5[:, 4, :])
    nc.vector.reciprocal(rs[:], ssum[:])
    nc.vector.tensor_mul(
        c2n[:], c2[:], rs[:].unsqueeze(1).broadcast_to([128, O, IB])
    )
    # scatter diag to CBD[(b,il), (o, b'=b), ib] (ib-contiguous runs)
    for b in range(BC):
        dmaqs[b % 3].dma_start(
            out=cbd[b * 16:(b + 1) * 16, b:80:8, :],
            in_=c2n[b * 16:(b + 1) * 16, :, :],
        )


_CACHE = {}


def _get(n_routing: int):
    if n_routing not in _CACHE:
        _CACHE[n_routing] = _build(n_routing)
    return _CACHE[n_routing]


def _bf16(a):
    import ml_dtypes

    return np.asarray(a, dtype=ml_dtypes.bfloat16)


def _prep_host(inputs: np.ndarray, W: np.ndarray):
    x = np.ascontiguousarray(np.asarray(inputs, dtype=np.float32))
    W = np.asarray(W, dtype=np.float32)
    # w2[(il,v), ib, (o,n)] = W[ib*16+il, o, v, n]
    w2 = np.ascontiguousarray(
        W.reshape(IB, 16, O, V, N).transpose(1, 3, 0, 2, 4).reshape(128, IB, ON)
    )
    # e2[(b,il), (o,b')] = 0.1 * (b == b')   (uniform softmax weights)
    e2 = np.zeros((128, 80), dtype=np.float32)
    for b in range(8):
        e2[b * 16:(b + 1) * 16, np.arange(O) * 8 + b] = 0.1
    return x, _bf16(w2), _bf16(e2)


def _make_in_maps(inputs, W):
    x, w2, e2 = _prep_host(inputs, W)
    in_maps = []
    for core in range(NCORES):
        xc = x[core * BLOC:(core + 1) * BLOC]              # [32, 1152, 8]
        # xbdh[c, il*8+v, ib, b*16+il] = xc[c*BC+b, ib*16+il, v]
        xr = xc.reshape(NCHUNK, BC, IB, 16, V)
        xbdh = np.zeros((NCHUNK, 128, IB, 128), dtype=np.float32)
        for il in range(16):
            xbdh[:, il * 8:(il + 1) * 8, :, il::16] = xr[:, :, :, il, :].transpose(
                0, 3, 2, 1
            )
        in_maps.append({"xbdh": _bf16(xbdh), "w2": w2, "e2": e2})
    return in_maps


def kernel(inputs, W, n_routing):
    from concourse.bass_utils import run_bass_kernel_spmd

    n_routing = int(n_routing)
    nc = _get(n_routing)
    in_maps = _make_in_maps(inputs, W)
    res = run_bass_kernel_spmd(nc, in_maps, core_ids=list(range(NCORES)))
    outs = [res.results[i]["out"] for i in range(NCORES)]
    return np.concatenate(outs, axis=0).astype(np.float32)
